# revision 2
# baseline (speedup 1.0000x reference)
"""Trainium2 Bass kernel for nn_Conv2d: x[32,128,56,56] * W[256,128,3,3] + b -> [32,256,56,56].

Stride 1, padding 1, dilation 1. Data-parallel over batch across 8 NeuronCores
(4 images per core, no collectives). Per core the conv is computed as 9
accumulated matmuls per output tile (one per kernel tap): PSUM[cout_chunk, R*56]
+= Wt[kh,kw][cin, cout_chunk].T-style matmul with a shifted window of a
zero-padded input image held in SBUF as [cin=128, 58, 58].

Self-contained: hardcodes shapes; host-side pre-transposes W to [cin, 9, cout]
so all device DMAs are contiguous.
"""

import numpy as np

B, CIN, H, W_ = 32, 128, 56, 56
COUT, KH, KW = 256, 3, 3
NCORES = 8
BPC = B // NCORES          # images per core
R = 8                      # output rows per tile -> matmul free dim R*56 = 448
NT = H // R                # row tiles per image
NPIX = R * W_              # 448
HP, WP = H + 2, W_ + 2     # padded 58x58

# "float32" = exact fp32 (4 cycles/row on PE). "float32r" = TF32-like
# single-pass mode (1 cycle/row at N>=256, ~1e-4 absmax relative error).
MM_DTYPE = "float32"

_cache = {}


def _build(mm_dtype_name):
    import concourse.mybir as mybir
    import concourse.tile as tile
    from concourse import bacc

    dt = mybir.dt
    mmdt = getattr(dt, mm_dtype_name)

    nc = bacc.Bacc("TRN2", target_bir_lowering=False, debug=False)

    x_d = nc.dram_tensor("x", [BPC, CIN, H, W_], mmdt, kind="ExternalInput")
    wt_d = nc.dram_tensor("wt", [CIN, KH * KW, COUT], mmdt, kind="ExternalInput")
    b_d = nc.dram_tensor("bias", [128, COUT // 128], dt.float32, kind="ExternalInput")
    o_d = nc.dram_tensor("out", [BPC, COUT, H, W_], dt.float32, kind="ExternalOutput")

    with tile.TileContext(nc) as tc:
        with (
            tc.tile_pool(name="const", bufs=1) as const_pool,
            tc.tile_pool(name="xin", bufs=1) as xin_pool,
            tc.tile_pool(name="outp", bufs=4) as out_pool,
            tc.tile_pool(name="psum", bufs=4, space="PSUM") as psum_pool,
        ):
            w_t = const_pool.tile([CIN, KH * KW, COUT], mmdt)
            nc.sync.dma_start(w_t[:], wt_d[:])
            b_t = const_pool.tile([128, COUT // 128], dt.float32)
            nc.sync.dma_start(b_t[:], b_d[:])

            # All 4 padded images resident in SBUF: 4*58*58*4B = 53.8KB/partition.
            xp = xin_pool.tile([CIN, BPC, HP, WP], mmdt)
            for n in range(BPC):
                nc.any.memset(xp[:, n, 0, :], 0.0)
                nc.any.memset(xp[:, n, HP - 1, :], 0.0)
                nc.any.memset(xp[:, n, :, 0], 0.0)
                nc.any.memset(xp[:, n, :, WP - 1], 0.0)
                nc.sync.dma_start(xp[:, n, 1 : H + 1, 1 : W_ + 1], x_d[n])

            for n in range(BPC):
                for ht in range(NT):
                    h0 = ht * R
                    for c in range(COUT // 128):
                        p = psum_pool.tile([128, R, W_], dt.float32, tag="ps")
                        for kh in range(KH):
                            for kw in range(KW):
                                pos = kh * KW + kw
                                nc.tensor.matmul(
                                    p[:],
                                    w_t[:, pos, c * 128 : (c + 1) * 128],
                                    xp[:, n, h0 + kh : h0 + kh + R, kw : kw + W_],
                                    start=(pos == 0),
                                    stop=(pos == KH * KW - 1),
                                )
                        ot = out_pool.tile([128, R, W_], dt.float32, tag="ot")
                        nc.scalar.activation(
                            ot[:],
                            p[:],
                            mybir.ActivationFunctionType.Identity,
                            bias=b_t[:, c : c + 1],
                        )
                        nc.sync.dma_start(
                            o_d[n, c * 128 : (c + 1) * 128, h0 : h0 + R, :], ot[:]
                        )

    nc.compile()
    return nc


def kernel(x, W, b):
    from concourse.bass_utils import run_bass_kernel_spmd

    if MM_DTYPE not in _cache:
        _cache[MM_DTYPE] = _build(MM_DTYPE)
    nc = _cache[MM_DTYPE]

    x = np.asarray(x, dtype=np.float32)
    W = np.asarray(W, dtype=np.float32)
    b = np.asarray(b, dtype=np.float32)

    # [cout, cin, kh, kw] -> [cin, kh*kw, cout], contiguous
    wt = np.ascontiguousarray(W.transpose(1, 2, 3, 0)).reshape(CIN, KH * KW, COUT)
    bh = np.ascontiguousarray(b.reshape(COUT // 128, 128).T)

    in_maps = [
        {
            "x": np.ascontiguousarray(x[core * BPC : (core + 1) * BPC]),
            "wt": wt,
            "bias": bh,
        }
        for core in range(NCORES)
    ]
    res = run_bass_kernel_spmd(nc, in_maps, list(range(NCORES))).results
    return np.concatenate([res[i]["out"] for i in range(NCORES)], axis=0)


# revision 4
# speedup vs baseline: 1.0145x; 1.0145x over previous
"""Trainium2 Bass kernel for nn_Conv2d: x[32,128,56,56] * W[256,128,3,3] + b -> [32,256,56,56].

Stride 1, padding 1, dilation 1. Data-parallel over batch across 8 NeuronCores
(4 images per core, no collectives). Per core the conv is computed as 9
accumulated matmuls per output tile (one per kernel tap): PSUM[cout_chunk, R*56]
+= Wt[kh,kw][cin, cout_chunk].T-style matmul with a shifted window of a
zero-padded input image held in SBUF as [cin=128, 58, 58].

Self-contained: hardcodes shapes; host-side pre-transposes W to [cin, 9, cout]
so all device DMAs are contiguous.
"""

import numpy as np

B, CIN, H, W_ = 32, 128, 56, 56
COUT, KH, KW = 256, 3, 3
NCORES = 8
BPC = B // NCORES          # images per core
R = 8                      # output rows per tile -> matmul free dim R*56 = 448
NT = H // R                # row tiles per image
NPIX = R * W_              # 448
HP, WP = H + 2, W_ + 2     # padded 58x58

# "float32" = exact fp32 (4 cycles/row on PE). "float32r" = TF32-like
# single-pass mode (1 cycle/row at N>=256, ~1e-4 absmax relative error).
MM_DTYPE = "float32"

_cache = {}


def _build(mm_dtype_name):
    import concourse.mybir as mybir
    import concourse.tile as tile
    from concourse import bacc

    dt = mybir.dt
    mmdt = getattr(dt, mm_dtype_name)

    nc = bacc.Bacc("TRN2", target_bir_lowering=False, debug=False)

    x_d = nc.dram_tensor("x", [BPC, CIN, H, W_], mmdt, kind="ExternalInput")
    wt_d = nc.dram_tensor("wt", [CIN, KH * KW, COUT], mmdt, kind="ExternalInput")
    b_d = nc.dram_tensor("bias", [128, COUT // 128], dt.float32, kind="ExternalInput")
    o_d = nc.dram_tensor("out", [BPC, COUT, H, W_], dt.float32, kind="ExternalOutput")

    with tile.TileContext(nc) as tc:
        with (
            tc.tile_pool(name="const", bufs=1) as const_pool,
            tc.tile_pool(name="xin", bufs=1) as xin_pool,
            tc.tile_pool(name="outp", bufs=4) as out_pool,
            tc.tile_pool(name="psum", bufs=4, space="PSUM") as psum_pool,
        ):
            # Split the weight DMA by cout chunk so chunk-0 matmuls can start
            # before the full 1.2MB of weights has landed.
            w_t = const_pool.tile([CIN, KH * KW, COUT], mmdt)
            for c in range(COUT // 128):
                nc.sync.dma_start(
                    w_t[:, :, c * 128 : (c + 1) * 128],
                    wt_d[:, :, c * 128 : (c + 1) * 128],
                )
            b_t = const_pool.tile([128, COUT // 128], dt.float32)
            nc.sync.dma_start(b_t[:], b_d[:])

            # One input tile per (image, row-tile): rows h0-1..h0+R of the
            # padded image (R+2 rows x 58 cols). Separate logical tiles keep
            # Tile's dependency tracking fine-grained: the first matmul group
            # only waits on its own ~290KB DMA, not all of x. Halo rows are
            # re-read from DRAM (25% extra x traffic; DMA is far from the
            # bottleneck). All BPC*NT tiles stay resident (~63KB/partition).
            xt = {}
            for n in range(BPC):
                for ht in range(NT):
                    h0 = ht * R
                    t = xin_pool.tile([CIN, R + 2, WP], mmdt, tag=f"x{n}_{ht}")
                    xt[(n, ht)] = t
                    nc.any.memset(t[:, :, 0], 0.0)
                    nc.any.memset(t[:, :, WP - 1], 0.0)
                    # interior rows present in DRAM: x rows [h0-1, h0+R]
                    r_lo = h0 - 1
                    r_hi = h0 + R  # inclusive
                    if r_lo < 0:
                        nc.any.memset(t[:, 0, :], 0.0)
                        r_lo = 0
                    if r_hi > H - 1:
                        nc.any.memset(t[:, R + 1, :], 0.0)
                        r_hi = H - 1
                    nc.sync.dma_start(
                        t[:, r_lo - (h0 - 1) : r_hi - (h0 - 1) + 1, 1 : W_ + 1],
                        x_d[n, :, r_lo : r_hi + 1, :],
                    )

            for n in range(BPC):
                for ht in range(NT):
                    t = xt[(n, ht)]
                    for c in range(COUT // 128):
                        p = psum_pool.tile([128, R, W_], dt.float32, tag="ps")
                        for kh in range(KH):
                            for kw in range(KW):
                                pos = kh * KW + kw
                                nc.tensor.matmul(
                                    p[:],
                                    w_t[:, pos, c * 128 : (c + 1) * 128],
                                    t[:, kh : kh + R, kw : kw + W_],
                                    start=(pos == 0),
                                    stop=(pos == KH * KW - 1),
                                )
                        ot = out_pool.tile([128, R, W_], dt.float32, tag="ot")
                        nc.scalar.activation(
                            ot[:],
                            p[:],
                            mybir.ActivationFunctionType.Identity,
                            bias=b_t[:, c : c + 1],
                        )
                        nc.sync.dma_start(
                            o_d[n, c * 128 : (c + 1) * 128, ht * R : ht * R + R, :],
                            ot[:],
                        )

    nc.compile()
    return nc


def kernel(x, W, b):
    from concourse.bass_utils import run_bass_kernel_spmd

    if MM_DTYPE not in _cache:
        _cache[MM_DTYPE] = _build(MM_DTYPE)
    nc = _cache[MM_DTYPE]

    x = np.asarray(x, dtype=np.float32)
    W = np.asarray(W, dtype=np.float32)
    b = np.asarray(b, dtype=np.float32)

    # [cout, cin, kh, kw] -> [cin, kh*kw, cout], contiguous
    wt = np.ascontiguousarray(W.transpose(1, 2, 3, 0)).reshape(CIN, KH * KW, COUT)
    bh = np.ascontiguousarray(b.reshape(COUT // 128, 128).T)

    in_maps = [
        {
            "x": np.ascontiguousarray(x[core * BPC : (core + 1) * BPC]),
            "wt": wt,
            "bias": bh,
        }
        for core in range(NCORES)
    ]
    res = run_bass_kernel_spmd(nc, in_maps, list(range(NCORES))).results
    return np.concatenate([res[i]["out"] for i in range(NCORES)], axis=0)
